# revision 4
# baseline (speedup 1.0000x reference)
"""BatchHardTripletLoss kernel for 8 Trainium2 NeuronCores.

Math (matches the jax reference):
  dist2[i,j] = |e1_i|^2 + |e2_j|^2 - 2 e1.e2 + 2*eps*(s1_i - s2_j) + D*eps^2
             = a[i] + v[i,j],   v[i,j] = b[j] - 2<e1_i, e2_j>
  pos_max[i] = sqrt(clip(a[i] + max_{j in pos} v[i,j], 0))
  neg_min[i] = sqrt(clip(a[i] + min_{j in neg} v[i,j], 0))
  loss = mean over POS anchors of relu(pos_max - neg_min + margin)

v2 architecture: PAIRWISE TOURNAMENT.  The drain of the [anchors x
cands] f32 distance matrix out of PSUM (DVE/Act at ~1 elem/cyc/
partition) was the wall in v1.  Here candidates are paired on the
host; for pair (u, v):   max(d_u, d_v) = d_v + relu(d_u - d_v)
and d_u - d_v = (b_u - b_v) - 2<e1, e2_u - e2_v> is itself ONE matmul
column.  Per PSUM group of 1024 pair-columns:
  phase A: K=2 bf16 bias-diff tails (start=True) + fp8 diff mains
  Act:     relu in-place on the PSUM group (PE never clears
           has_written, so the later accumulate still works)
  phase B: K=2 bf16 base-bias tails + fp8 base mains, all
           start=False -> accumulate d_v on top of relu(d_u - d_v)
  DVE:     one chained tensor_scalar max-accum per class segment
This HALVES the reduced stream (4096 pair-cols vs 8192 cols per
i-tile).  Neg class is sign-flipped so both classes are MAX chains.
4 PSUM groups (2 banks each) rotate; PE/Act/DVE pipeline across them.

Host: pos-first column sort, exact f64 row stats, pairing (self-pair
for odd class tails, -BIG dummy pad to 4096 pairs, odd columns peeled
to an exact host-side merge), fp8/bf16 packing, final sqrt/margin/mean
+ exact f64 remainder rows.
"""

import os
import sys

for _p in ("/opt/trn_rl_repo",):
    if _p not in sys.path:
        sys.path.insert(0, _p)

import numpy as np
import ml_dtypes

EPS = 1e-6
MARGIN = 0.2
B = 8192
D = 128
NCORES = 8
NPAIR = 4096          # pair-columns per core (all cores see all pairs)
GW = 1024             # pair-cols per PSUM group = 2 banks
NG = NPAIR // GW      # 4 groups per i-tile
BIG = 1.0e30

_programs = {}
LAST_RESULTS = None   # BassKernelResults of the most recent run (for profiling)


def _build_program(n_it: int, pairb: int):
    """Bass program for one core.

    n_it: i-tiles (of 128 anchors) per core.
    pairb: pos/neg boundary in pair-column space (pairs [0,pairb) are
      pos-class, [pairb, NPAIR) neg-class).
    """
    import concourse.bacc as bacc
    import concourse.tile as tile
    from concourse import mybir

    f32 = mybir.dt.float32
    bf16 = mybir.dt.bfloat16
    fp8 = mybir.dt.float8e4
    AOT = mybir.AluOpType
    AFT = mybir.ActivationFunctionType

    SH = n_it * 128

    nc = bacc.Bacc(None)
    e1t = nc.declare_dram_parameter("e1t", [D, SH], fp8, isOutput=False)
    rhsA = nc.declare_dram_parameter("rhsA", [D, NPAIR], fp8, isOutput=False)
    rhsB = nc.declare_dram_parameter("rhsB", [D, NPAIR], fp8, isOutput=False)
    # tails rows 0..3: biasA hi,lo,hi,lo ; rows 4..7: biasB hi,lo,hi,lo
    # cols [0:SH] = lhsT ones, [SH:] = bias values per pair-col.
    tails = nc.declare_dram_parameter("tails", [8, SH + NPAIR], bf16, isOutput=False)
    outp = nc.declare_dram_parameter("out", [128, 2 * n_it], f32, isOutput=True)

    def group_segs(g):
        """Class segments (lo, hi, is_pos) of group g in pair-col coords."""
        glo, ghi = g * GW, (g + 1) * GW
        segs = []
        if glo < pairb:
            segs.append((glo, min(ghi, pairb), True))
        if ghi > pairb:
            segs.append((max(glo, pairb), ghi, False))
        return segs

    with tile.TileContext(nc) as tc:
        with (
            tc.tile_pool(name="const", bufs=1) as cpool,
            tc.tile_pool(name="ps", bufs=4, space="PSUM") as pspool,
            tc.tile_pool(name="red", bufs=2) as redpool,
        ):
            # DMA routing: keep the Act queue FREE (it runs the relus).
            # sync queue: e1, tails strips (gen-0-critical), rhsB chunks.
            # gpsimd queue: rhsA chunks + bulk.
            tlsb = cpool.tile([128, SH + NPAIR], bf16, tag="tlsb")
            e1sb = cpool.tile([D, SH], fp8, tag="e1sb")
            outsb = cpool.tile([128, 2 * n_it], f32, tag="outsb")
            trf = cpool.tile([128, GW], bf16, tag="trf")
            rhsAsb = cpool.tile([D, NPAIR], fp8, tag="rhsAsb")
            rhsBsb = cpool.tile([D, NPAIR], fp8, tag="rhsBsb")

            nc.sync.dma_start(e1sb[:], e1t[:])
            nc.gpsimd.dma_start(
                rhsAsb[:, 0:GW], rhsA[:, 0:GW]
            )
            # tails: strip s lands on partitions 32s..32s+1; first half
            # (ones + early bias cols) on sync, rest on gpsimd.
            half = (SH + NPAIR) // 2
            for s in range(4):
                nc.sync.dma_start(
                    tlsb[32 * s:32 * s + 2, 0:half], tails[2 * s:2 * s + 2, 0:half]
                )
            nc.sync.dma_start(rhsBsb[:, 0:GW], rhsB[:, 0:GW])
            for s in range(4):
                nc.gpsimd.dma_start(
                    tlsb[32 * s:32 * s + 2, half:], tails[2 * s:2 * s + 2, half:]
                )
            # remaining chunks so later groups stream in behind gen 0
            for g in range(1, NG):
                nc.gpsimd.dma_start(
                    rhsAsb[:, g * GW:(g + 1) * GW], rhsA[:, g * GW:(g + 1) * GW]
                )
                nc.sync.dma_start(
                    rhsBsb[:, g * GW:(g + 1) * GW], rhsB[:, g * GW:(g + 1) * GW]
                )

            def emit_A(it, g, ps):
                icols = slice(it * 128, (it + 1) * 128)
                gcol = g * GW
                # bias-diff tails (start=True) + fp8 diff mains
                for s in range(2):
                    j0 = SH + gcol + s * 512
                    nc.tensor.matmul(
                        ps[:, s * 512:(s + 1) * 512],
                        tlsb[32 * s:32 * s + 2, icols],
                        tlsb[32 * s:32 * s + 2, j0:j0 + 512],
                        start=True,
                        stop=False,
                        tile_position=(32 * s, 0),
                    )
                for s in range(2):
                    nc.tensor.matmul(
                        ps[:, s * 512:(s + 1) * 512],
                        e1sb[:, icols],
                        rhsAsb[:, gcol + s * 512:gcol + (s + 1) * 512],
                        start=False,
                        stop=True,
                    )
                # relu in place (PSUM -> PSUM, has_written untouched)
                nc.scalar.activation(ps[:], ps[:], AFT.Relu)

            def emit_B(it, g, ps, chain, chain_used):
                icols = slice(it * 128, (it + 1) * 128)
                gcol = g * GW
                # base tails + fp8 base mains, accumulate onto relu
                for s in range(2):
                    j0 = SH + gcol + s * 512
                    nc.tensor.matmul(
                        ps[:, s * 512:(s + 1) * 512],
                        tlsb[64 + 32 * s:64 + 32 * s + 2, icols],
                        tlsb[64 + 32 * s:64 + 32 * s + 2, j0:j0 + 512],
                        start=False,
                        stop=False,
                        tile_position=(64 + 32 * s, 0),
                        skip_group_check=True,
                    )
                for s in range(2):
                    nc.tensor.matmul(
                        ps[:, s * 512:(s + 1) * 512],
                        e1sb[:, icols],
                        rhsBsb[:, gcol + s * 512:gcol + (s + 1) * 512],
                        start=False,
                        stop=True,
                        skip_group_check=True,
                    )
                # drain: chained max-accum per class segment
                for lo, hi, is_pos in group_segs(g):
                    ll, lh = lo - gcol, hi - gcol
                    ci = 0 if is_pos else 1
                    nc.vector.tensor_scalar(
                        out=trf[:, ll:lh],
                        in0=ps[:, ll:lh],
                        scalar1=(chain[:, ci:ci + 1]
                                 if chain_used[is_pos] else -BIG),
                        scalar2=None,
                        op0=AOT.max,
                        op1=AOT.max,
                        accum_out=chain[:, ci:ci + 1],
                    )
                    chain_used[is_pos] = True
                if g == NG - 1:
                    nc.vector.tensor_copy(outsb[:, 2 * it:2 * it + 2], chain[:])

            # Software pipeline: emit A(gen n+1) BEFORE B(gen n), so the
            # PE FIFO never head-of-line blocks on the relu of gen n.
            chains = {}
            pending = None
            for it in range(n_it):
                chains[it] = (
                    redpool.tile([128, 2], f32, tag="chain", name=f"chain_{it}"),
                    {True: False, False: False},
                )
                for g in range(NG):
                    ps = pspool.tile([128, GW], f32, tag="ps", name=f"ps_{it}_{g}")
                    emit_A(it, g, ps)
                    if pending is not None:
                        pit, pg, pps = pending
                        emit_B(pit, pg, pps, *chains[pit])
                    pending = (it, g, ps)
            pit, pg, pps = pending
            emit_B(pit, pg, pps, *chains[pit])
            nc.sync.dma_start(outp[:], outsb[:])
    nc.compile()
    return nc


def _host_prep(emb1, emb2, target):
    """Sort columns pos-first, build pairs, pack device operands.

    Returns (k, n_it, a, e1p, pairb, e1t8, rhsA8, rhsB8, tails, peeled)
    peeled: list of (col_vector_f64, bias_f64, is_pos) handled on host.
    """
    tpos = target == 1
    k = int(tpos.sum())
    perm = np.concatenate([np.nonzero(tpos)[0], np.nonzero(~tpos)[0]])
    e2s = emb2[perm].astype(np.float64)          # [B, D] sorted pos-first
    b = (e2s * e2s).sum(1) - (2.0 * EPS) * e2s.sum(1)

    nneg = B - k
    peel_pos = k % 2
    peel_neg = nneg % 2
    k2, n2 = k - peel_pos, nneg - peel_neg
    peeled = []
    if peel_pos:
        peeled.append((e2s[k - 1], b[k - 1], True))
    if peel_neg:
        peeled.append((e2s[B - 1], b[B - 1], False))

    npairs_pos = k2 // 2
    npairs_neg = n2 // 2
    ndum = NPAIR - npairs_pos - npairs_neg
    assert ndum >= 0

    # pair columns: pos pairs (u=2p, v=2p+1), then neg pairs, then dummies
    dA = np.zeros((NPAIR, D))                    # rhsA columns (diff side)
    dB = np.zeros((NPAIR, D))                    # rhsB columns (base side)
    bA = np.zeros(NPAIR)
    bB = np.full(NPAIR, -BIG)                    # dummies default -BIG
    # pos: A = d_u - d_v ; B = d_v
    u = e2s[0:k2:2]
    v = e2s[1:k2:2]
    dA[:npairs_pos] = u - v
    dB[:npairs_pos] = v
    bA[:npairs_pos] = b[0:k2:2] - b[1:k2:2]
    bB[:npairs_pos] = b[1:k2:2]
    # neg (sign-flipped): A = d_v - d_u ; B = -d_v
    nu = e2s[k:k + n2:2]
    nv = e2s[k + 1:k + n2:2]
    sl = slice(npairs_pos, npairs_pos + npairs_neg)
    dA[sl] = nv - nu
    dB[sl] = -nv
    bA[sl] = b[k + 1:k + n2:2] - b[k:k + n2:2]
    bB[sl] = -b[k + 1:k + n2:2]

    e1p = emb1[tpos]                             # [k, D] pos anchors
    e1d = e1p.astype(np.float64)
    a = (e1d * e1d).sum(1) + (2.0 * EPS) * e1d.sum(1) + D * EPS * EPS

    n_it = min(k // 1024, 8)
    ndev = n_it * 1024
    e1dev = e1p[:ndev]

    e1m2t = np.ascontiguousarray((-2.0 * e1dev).T)      # [D, ndev] f32
    e1t8 = e1m2t.astype(ml_dtypes.float8_e4m3)
    rhsA8 = np.ascontiguousarray(dA.T).astype(np.float32).astype(
        ml_dtypes.float8_e4m3)
    rhsB8 = np.ascontiguousarray(dB.T).astype(np.float32).astype(
        ml_dtypes.float8_e4m3)

    SH = n_it * 128
    tails = np.zeros((8, SH + NPAIR), dtype=ml_dtypes.bfloat16)
    tails[:, 0:SH] = 1.0
    for src, base in ((bA, 0), (bB, 4)):
        hi = src.astype(np.float32).astype(ml_dtypes.bfloat16)
        lo = (src.astype(np.float32) - hi.astype(np.float32)).astype(
            ml_dtypes.bfloat16)
        for s in range(2):
            tails[base + 2 * s + 0, SH:] = hi
            tails[base + 2 * s + 1, SH:] = lo
    pairb = npairs_pos
    return k, n_it, a, e1p, pairb, e1t8, rhsA8, rhsB8, tails, peeled


def _host_remainder(e1rem, emb2, target):
    """Exact f64 pos_max/neg_min contribution of the remainder anchors."""
    e1d = e1rem.astype(np.float64)
    e2d = emb2.astype(np.float64)
    sq = (
        (e1d * e1d).sum(1)[:, None]
        + (e2d * e2d).sum(1)[None, :]
        - 2.0 * (e1d @ e2d.T)
        + 2.0 * EPS * (e1d.sum(1)[:, None] - e2d.sum(1)[None, :])
        + D * EPS * EPS
    )
    dist = np.sqrt(np.clip(sq, 0.0, None))
    pos = target == 1
    pos_max = np.where(pos[None, :], dist, -np.inf).max(1)
    neg_min = np.where(~pos[None, :], dist, np.inf).min(1)
    return np.clip(pos_max - neg_min + MARGIN, 0.0, None).sum()


def _numpy_fallback(emb1, emb2, target):
    e1 = emb1.astype(np.float64)
    e2 = emb2.astype(np.float64)
    sq = (
        (e1 * e1).sum(1)[:, None]
        + (e2 * e2).sum(1)[None, :]
        - 2.0 * (e1 @ e2.T)
        + 2.0 * EPS * (e1.sum(1)[:, None] - e2.sum(1)[None, :])
        + D * EPS * EPS
    )
    dist = np.sqrt(np.clip(sq, 0.0, None))
    pos = target == 1
    neg = target == 0
    pos_max = np.where(pos[None, :], dist, -np.inf).max(1)
    neg_min = np.where(neg[None, :], dist, np.inf).min(1)
    per = np.maximum(pos_max - neg_min + MARGIN, 0.0)
    w = pos.astype(np.float64)
    return np.float32((per * w).sum() / w.sum())


def kernel(emb1, emb2, target):
    global LAST_RESULTS
    emb1 = np.asarray(emb1, dtype=np.float32)
    emb2 = np.asarray(emb2, dtype=np.float32)
    target = np.asarray(target)
    assert emb1.shape == (B, D) and emb2.shape == (B, D)

    k = int((target == 1).sum())
    if k < 1024 or k == B:
        return _numpy_fallback(emb1, emb2, target)

    (k, n_it, a, e1p, pairb, e1t8, rhsA8, rhsB8, tails,
     peeled) = _host_prep(emb1, emb2, target)
    ndev = n_it * 1024
    SH = n_it * 128

    nc = _programs.get((n_it, pairb))
    if nc is None:
        nc = _build_program(n_it, pairb)
        _programs[(n_it, pairb)] = nc

    from concourse.bass_utils import run_bass_kernel_spmd

    in_maps = [
        {
            "e1t": np.ascontiguousarray(e1t8[:, c * SH:(c + 1) * SH]),
            "rhsA": rhsA8,
            "rhsB": rhsB8,
            "tails": tails,
        }
        for c in range(NCORES)
    ]
    res = run_bass_kernel_spmd(nc, in_maps, core_ids=list(range(NCORES)))
    LAST_RESULTS = res

    Mp = np.concatenate(
        [np.asarray(res.results[c]["out"])[:, 0::2].T.reshape(-1)
         for c in range(NCORES)]
    ).astype(np.float64)
    Mn = np.concatenate(
        [np.asarray(res.results[c]["out"])[:, 1::2].T.reshape(-1)
         for c in range(NCORES)]
    ).astype(np.float64)

    # merge peeled columns exactly (host f64)
    e1d = e1p[:ndev].astype(np.float64)
    for col, bias, is_pos in peeled:
        vj = bias - 2.0 * (e1d @ col)
        if is_pos:
            Mp = np.maximum(Mp, vj)
        else:
            Mn = np.maximum(Mn, -vj)

    adev = a[:ndev]
    pos2 = np.clip(adev + Mp, 0.0, None)
    neg2 = np.clip(adev - Mn, 0.0, None)   # min v = -max(-v)
    per = np.clip(np.sqrt(pos2) - np.sqrt(neg2) + MARGIN, 0.0, None)
    total = per.sum()
    if ndev < k:
        total += _host_remainder(e1p[ndev:], emb2, target)
    return np.float32(total / k)


# revision 5
# speedup vs baseline: 1.3812x; 1.3812x over previous
"""BatchHardTripletLoss kernel for 8 Trainium2 NeuronCores.

Math (matches the jax reference):
  dist2[i,j] = |e1_i|^2 + |e2_j|^2 - 2 e1.e2 + 2*eps*(s1_i - s2_j) + D*eps^2
             = a[i] + v[i,j],   v[i,j] = b[j] - 2<e1_i, e2_j>
  pos_max[i] = sqrt(clip(a[i] + max_{j in pos} v[i,j], 0))
  neg_min[i] = sqrt(clip(a[i] + min_{j in neg} v[i,j], 0))
  loss = mean over POS anchors of relu(pos_max - neg_min + margin)

v3 architecture: PAIRWISE TOURNAMENT + DoubleRow bias folding.
The drain of the [anchors x cands] f32 matrix out of PSUM (DVE/Act at
~1 elem/cyc/partition) is the wall, so candidates are paired on the
host:  max(d_u, d_v) = d_v + relu(d_u - d_v), and d_u - d_v =
(b_u - b_v) - 2<e1, e2_u - e2_v> is ONE matmul column.  Per PSUM
group of 1024 pair-columns:
  phase A: fp8 DoubleRow mains (virtual K=256: 128 embedding dims on
           the i=0 plane, 4-term fp8 bias split on rows 0-3 of the
           i=1 plane against ones in the lhsT) -> diff + bias-diff
  Act:     relu in-place on the PSUM group (only TensorE touches
           has_written, so the accumulate below still works)
  phase B: fp8 DoubleRow mains, start=False -> accumulate base
           d_v (+ its bias) on top of relu(d_u - d_v)
  DVE:     one chained tensor_scalar max-accum per class segment
This halves the reduced stream (4096 pair-cols vs 8192 cols per
i-tile).  Neg class is sign-flipped so both classes are MAX chains.
A-phases are emitted one group ahead of B-phases so the PE FIFO never
head-of-line blocks on a relu.  4 PSUM groups (2 banks each) rotate.

Host: pos-first sort, exact f64 stats, pairing (self-pair for odd
class tails, fp8-saturated -BIG dummy pad to 4096 pairs, odd columns
peeled into an exact host-side merge), packing, final sqrt/margin/mean
+ exact f64 remainder rows.

Operand layouts (fp8 e4m3):
  e1dr [128, n_it*256]: per i-tile block of 256 cols: [0:128] =
    (-2*e1).T anchor block, [128:256] = ones on partitions 0-3.
  rhsA/rhsB [128, 8192]: per (g, s) chunk of 1024 cols at
    g*2048 + s*1024: [0:512] = pair columns (diff / base side),
    [512:1024] = bias plane (rows 0-3 = 4-term fp8 bias split).
"""

import os
import sys

for _p in ("/opt/trn_rl_repo",):
    if _p not in sys.path:
        sys.path.insert(0, _p)

import numpy as np
import ml_dtypes

EPS = 1e-6
MARGIN = 0.2
B = 8192
D = 128
NCORES = 8
NPAIR = 4096          # pair-columns per core (all cores see all pairs)
GW = 1024             # pair-cols per PSUM group = 2 banks
NG = NPAIR // GW      # 4 groups per i-tile
BIG = 1.0e30

_programs = {}
LAST_RESULTS = None   # BassKernelResults of the most recent run (for profiling)


def _build_program(n_it: int, pairb: int):
    """Bass program for one core.

    n_it: i-tiles (of 128 anchors) per core.
    pairb: pos/neg boundary in pair-column space.
    """
    import concourse.bacc as bacc
    import concourse.tile as tile
    from concourse import mybir

    f32 = mybir.dt.float32
    bf16 = mybir.dt.bfloat16
    fp8 = mybir.dt.float8e4
    AOT = mybir.AluOpType
    AFT = mybir.ActivationFunctionType
    DR = mybir.MatmulPerfMode.DoubleRow

    SH = n_it * 128

    nc = bacc.Bacc(None)
    e1dr = nc.declare_dram_parameter("e1dr", [D, 2 * SH], fp8, isOutput=False)
    rhsA = nc.declare_dram_parameter("rhsA", [D, 2 * NPAIR], fp8, isOutput=False)
    rhsB = nc.declare_dram_parameter("rhsB", [D, 2 * NPAIR], fp8, isOutput=False)
    outp = nc.declare_dram_parameter("out", [128, 2 * n_it], f32, isOutput=True)

    def group_segs(g):
        """Class segments (lo, hi, is_pos) of group g in pair-col coords."""
        glo, ghi = g * GW, (g + 1) * GW
        segs = []
        if glo < pairb:
            segs.append((glo, min(ghi, pairb), True))
        if ghi > pairb:
            segs.append((max(glo, pairb), ghi, False))
        return segs

    with tile.TileContext(nc) as tc:
        with (
            tc.tile_pool(name="const", bufs=1) as cpool,
            tc.tile_pool(name="ps", bufs=4, space="PSUM") as pspool,
            tc.tile_pool(name="red", bufs=2) as redpool,
        ):
            e1sb = cpool.tile([D, 2 * SH], fp8, tag="e1sb")
            outsb = cpool.tile([128, 2 * n_it], f32, tag="outsb")
            trf = cpool.tile([128, GW], bf16, tag="trf")
            rhsAsb = cpool.tile([D, 2 * NPAIR], fp8, tag="rhsAsb")
            rhsBsb = cpool.tile([D, 2 * NPAIR], fp8, tag="rhsBsb")

            # DMA: Act queue stays free for relus.  Chunked per group so
            # group 0 starts early.
            nc.sync.dma_start(e1sb[:], e1dr[:])
            nc.gpsimd.dma_start(rhsAsb[:, 0:2048], rhsA[:, 0:2048])
            nc.sync.dma_start(rhsBsb[:, 0:2048], rhsB[:, 0:2048])
            for g in range(1, NG):
                nc.gpsimd.dma_start(
                    rhsAsb[:, g * 2048:(g + 1) * 2048],
                    rhsA[:, g * 2048:(g + 1) * 2048],
                )
                nc.sync.dma_start(
                    rhsBsb[:, g * 2048:(g + 1) * 2048],
                    rhsB[:, g * 2048:(g + 1) * 2048],
                )

            def emit_A(it, g, ps):
                w3 = e1sb[:, it * 256:(it + 1) * 256].rearrange(
                    "p (i m) -> p i m", i=2
                )
                for s in range(2):
                    c0 = g * 2048 + s * 1024
                    nc.tensor.matmul(
                        ps[:, s * 512:(s + 1) * 512],
                        w3,
                        rhsAsb[:, c0:c0 + 1024].rearrange(
                            "p (i n) -> p i n", i=2
                        ),
                        start=True,
                        stop=True,
                        perf_mode=DR,
                    )
                # relu in place (PSUM -> PSUM, has_written untouched)
                nc.scalar.activation(ps[:], ps[:], AFT.Relu)

            def emit_B(it, g, ps, chain, chain_used):
                w3 = e1sb[:, it * 256:(it + 1) * 256].rearrange(
                    "p (i m) -> p i m", i=2
                )
                for s in range(2):
                    c0 = g * 2048 + s * 1024
                    nc.tensor.matmul(
                        ps[:, s * 512:(s + 1) * 512],
                        w3,
                        rhsBsb[:, c0:c0 + 1024].rearrange(
                            "p (i n) -> p i n", i=2
                        ),
                        start=False,
                        stop=True,
                        perf_mode=DR,
                        skip_group_check=True,
                    )
                # drain: chained max-accum per class segment
                for lo, hi, is_pos in group_segs(g):
                    ll, lh = lo - g * GW, hi - g * GW
                    ci = 0 if is_pos else 1
                    nc.vector.tensor_scalar(
                        out=trf[:, ll:lh],
                        in0=ps[:, ll:lh],
                        scalar1=(chain[:, ci:ci + 1]
                                 if chain_used[is_pos] else -BIG),
                        scalar2=None,
                        op0=AOT.max,
                        op1=AOT.max,
                        accum_out=chain[:, ci:ci + 1],
                    )
                    chain_used[is_pos] = True
                if g == NG - 1:
                    nc.vector.tensor_copy(outsb[:, 2 * it:2 * it + 2], chain[:])

            # Software pipeline: A(gen n+1) is emitted BEFORE B(gen n) so
            # the PE FIFO never head-of-line blocks on the relu of gen n.
            chains = {}
            pending = None
            for it in range(n_it):
                chains[it] = (
                    redpool.tile([128, 2], f32, tag="chain", name=f"chain_{it}"),
                    {True: False, False: False},
                )
                for g in range(NG):
                    ps = pspool.tile([128, GW], f32, tag="ps", name=f"ps_{it}_{g}")
                    emit_A(it, g, ps)
                    if pending is not None:
                        pit, pg, pps = pending
                        emit_B(pit, pg, pps, *chains[pit])
                    pending = (it, g, ps)
            pit, pg, pps = pending
            emit_B(pit, pg, pps, *chains[pit])
            nc.sync.dma_start(outp[:], outsb[:])
    nc.compile()
    return nc


def _fp8_split4(x):
    """4-term fp8 e4m3 split of x (f64): returns [4, n] planes whose sum
    approximates x to ~1e-3 absolute (saturates at +-448*4)."""
    terms = []
    rem = x.astype(np.float64).copy()
    for _ in range(4):
        t = rem.astype(np.float32).astype(ml_dtypes.float8_e4m3)
        terms.append(t)
        rem = rem - t.astype(np.float64)
    return np.stack(terms)


def _host_prep(emb1, emb2, target):
    """Sort columns pos-first, build pairs, pack device operands."""
    tpos = target == 1
    k = int(tpos.sum())
    perm = np.concatenate([np.nonzero(tpos)[0], np.nonzero(~tpos)[0]])
    e2s = emb2[perm].astype(np.float64)          # [B, D] sorted pos-first
    b = (e2s * e2s).sum(1) - (2.0 * EPS) * e2s.sum(1)

    nneg = B - k
    peel_pos = k % 2
    peel_neg = nneg % 2
    k2, n2 = k - peel_pos, nneg - peel_neg
    peeled = []
    if peel_pos:
        peeled.append((e2s[k - 1], b[k - 1], True))
    if peel_neg:
        peeled.append((e2s[B - 1], b[B - 1], False))

    npairs_pos = k2 // 2
    npairs_neg = n2 // 2
    ndum = NPAIR - npairs_pos - npairs_neg
    assert ndum >= 0

    dA = np.zeros((NPAIR, D))                    # rhsA pair columns (diff)
    dB = np.zeros((NPAIR, D))                    # rhsB pair columns (base)
    bA = np.zeros(NPAIR)
    bB = np.full(NPAIR, -4.0 * 448.0)            # dummy: 4x fp8-saturated
    # pos: A = d_u - d_v ; B = d_v
    u = e2s[0:k2:2]
    v = e2s[1:k2:2]
    dA[:npairs_pos] = u - v
    dB[:npairs_pos] = v
    bA[:npairs_pos] = b[0:k2:2] - b[1:k2:2]
    bB[:npairs_pos] = b[1:k2:2]
    # neg (sign-flipped): A = d_v - d_u ; B = -d_v
    nu = e2s[k:k + n2:2]
    nv = e2s[k + 1:k + n2:2]
    sl = slice(npairs_pos, npairs_pos + npairs_neg)
    dA[sl] = nv - nu
    dB[sl] = -nv
    bA[sl] = b[k + 1:k + n2:2] - b[k:k + n2:2]
    bB[sl] = -b[k + 1:k + n2:2]

    e1p = emb1[tpos]                             # [k, D] pos anchors
    e1d = e1p.astype(np.float64)
    a = (e1d * e1d).sum(1) + (2.0 * EPS) * e1d.sum(1) + D * EPS * EPS

    n_it = min(k // 1024, 8)
    ndev = n_it * 1024
    SH = n_it * 128

    # e1dr: per-core built later (anchor blocks per core); build full here
    e1m2t = (-2.0 * e1p[:ndev]).T.astype(np.float32)   # [D, ndev]

    def pack_rhs(cols, bias):
        # [128, 2*NPAIR]: per (g, s) 1024-chunk: [0:512] cols, [512:1024] bias
        out = np.zeros((D, 2 * NPAIR), dtype=ml_dtypes.float8_e4m3)
        colsT = np.ascontiguousarray(cols.T).astype(np.float32).astype(
            ml_dtypes.float8_e4m3)                  # [D, NPAIR]
        bias4 = _fp8_split4(bias)                   # [4, NPAIR]
        for g in range(NG):
            for s in range(2):
                c0 = g * 2048 + s * 1024
                j0 = g * GW + s * 512
                out[:, c0:c0 + 512] = colsT[:, j0:j0 + 512]
                out[0:4, c0 + 512:c0 + 1024] = bias4[:, j0:j0 + 512]
        return out

    rhsA8 = pack_rhs(dA, bA)
    rhsB8 = pack_rhs(dB, bB)
    pairb = npairs_pos
    return k, n_it, a, e1p, pairb, e1m2t, rhsA8, rhsB8, peeled


def _pack_e1dr(e1m2t_core, n_it):
    """[D, SH] f32 -> [D, 2*SH] fp8 per-i-tile blocks [emb | ones-rows]."""
    SH = n_it * 128
    out = np.zeros((D, 2 * SH), dtype=ml_dtypes.float8_e4m3)
    for it in range(n_it):
        blk = e1m2t_core[:, it * 128:(it + 1) * 128]
        out[:, it * 256:it * 256 + 128] = blk.astype(ml_dtypes.float8_e4m3)
        out[0:4, it * 256 + 128:it * 256 + 256] = 1.0
    return out


def _host_remainder(e1rem, emb2, target):
    """Exact f64 pos_max/neg_min contribution of the remainder anchors."""
    e1d = e1rem.astype(np.float64)
    e2d = emb2.astype(np.float64)
    sq = (
        (e1d * e1d).sum(1)[:, None]
        + (e2d * e2d).sum(1)[None, :]
        - 2.0 * (e1d @ e2d.T)
        + 2.0 * EPS * (e1d.sum(1)[:, None] - e2d.sum(1)[None, :])
        + D * EPS * EPS
    )
    dist = np.sqrt(np.clip(sq, 0.0, None))
    pos = target == 1
    pos_max = np.where(pos[None, :], dist, -np.inf).max(1)
    neg_min = np.where(~pos[None, :], dist, np.inf).min(1)
    return np.clip(pos_max - neg_min + MARGIN, 0.0, None).sum()


def _numpy_fallback(emb1, emb2, target):
    e1 = emb1.astype(np.float64)
    e2 = emb2.astype(np.float64)
    sq = (
        (e1 * e1).sum(1)[:, None]
        + (e2 * e2).sum(1)[None, :]
        - 2.0 * (e1 @ e2.T)
        + 2.0 * EPS * (e1.sum(1)[:, None] - e2.sum(1)[None, :])
        + D * EPS * EPS
    )
    dist = np.sqrt(np.clip(sq, 0.0, None))
    pos = target == 1
    neg = target == 0
    pos_max = np.where(pos[None, :], dist, -np.inf).max(1)
    neg_min = np.where(neg[None, :], dist, np.inf).min(1)
    per = np.maximum(pos_max - neg_min + MARGIN, 0.0)
    w = pos.astype(np.float64)
    return np.float32((per * w).sum() / w.sum())


def kernel(emb1, emb2, target):
    global LAST_RESULTS
    emb1 = np.asarray(emb1, dtype=np.float32)
    emb2 = np.asarray(emb2, dtype=np.float32)
    target = np.asarray(target)
    assert emb1.shape == (B, D) and emb2.shape == (B, D)

    k = int((target == 1).sum())
    if k < 1024 or k == B:
        return _numpy_fallback(emb1, emb2, target)

    (k, n_it, a, e1p, pairb, e1m2t, rhsA8, rhsB8,
     peeled) = _host_prep(emb1, emb2, target)
    ndev = n_it * 1024
    SH = n_it * 128

    nc = _programs.get((n_it, pairb))
    if nc is None:
        nc = _build_program(n_it, pairb)
        _programs[(n_it, pairb)] = nc

    from concourse.bass_utils import run_bass_kernel_spmd

    in_maps = [
        {
            "e1dr": _pack_e1dr(e1m2t[:, c * SH:(c + 1) * SH], n_it),
            "rhsA": rhsA8,
            "rhsB": rhsB8,
        }
        for c in range(NCORES)
    ]
    res = run_bass_kernel_spmd(nc, in_maps, core_ids=list(range(NCORES)))
    LAST_RESULTS = res

    Mp = np.concatenate(
        [np.asarray(res.results[c]["out"])[:, 0::2].T.reshape(-1)
         for c in range(NCORES)]
    ).astype(np.float64)
    Mn = np.concatenate(
        [np.asarray(res.results[c]["out"])[:, 1::2].T.reshape(-1)
         for c in range(NCORES)]
    ).astype(np.float64)

    # merge peeled columns exactly (host f64)
    e1d = e1p[:ndev].astype(np.float64)
    for col, bias, is_pos in peeled:
        vj = bias - 2.0 * (e1d @ col)
        if is_pos:
            Mp = np.maximum(Mp, vj)
        else:
            Mn = np.maximum(Mn, -vj)

    adev = a[:ndev]
    pos2 = np.clip(adev + Mp, 0.0, None)
    neg2 = np.clip(adev - Mn, 0.0, None)   # min v = -max(-v)
    per = np.clip(np.sqrt(pos2) - np.sqrt(neg2) + MARGIN, 0.0, None)
    total = per.sum()
    if ndev < k:
        total += _host_remainder(e1p[ndev:], emb2, target)
    return np.float32(total / k)


# revision 11
# speedup vs baseline: 1.3830x; 1.0013x over previous
"""BatchHardTripletLoss kernel for 8 Trainium2 NeuronCores.

Math (matches the jax reference):
  dist2[i,j] = |e1_i|^2 + |e2_j|^2 - 2 e1.e2 + 2*eps*(s1_i - s2_j) + D*eps^2
             = a[i] + v[i,j],   v[i,j] = b[j] - 2<e1_i, e2_j>
  pos_max[i] = sqrt(clip(a[i] + max_{j in pos} v[i,j], 0))
  neg_min[i] = sqrt(clip(a[i] + min_{j in neg} v[i,j], 0))
  loss = mean over POS anchors of relu(pos_max - neg_min + margin)

v3 architecture: PAIRWISE TOURNAMENT + DoubleRow bias folding.
The drain of the [anchors x cands] f32 matrix out of PSUM (DVE/Act at
~1 elem/cyc/partition) is the wall, so candidates are paired on the
host:  max(d_u, d_v) = d_v + relu(d_u - d_v), and d_u - d_v =
(b_u - b_v) - 2<e1, e2_u - e2_v> is ONE matmul column.  Per PSUM
group of 1024 pair-columns:
  phase A: fp8 DoubleRow mains (virtual K=256: 128 embedding dims on
           the i=0 plane, 4-term fp8 bias split on rows 0-3 of the
           i=1 plane against ones in the lhsT) -> diff + bias-diff
  Act:     relu in-place on the PSUM group (only TensorE touches
           has_written, so the accumulate below still works)
  phase B: fp8 DoubleRow mains, start=False -> accumulate base
           d_v (+ its bias) on top of relu(d_u - d_v)
  DVE:     one chained tensor_scalar max-accum per class segment
This halves the reduced stream (4096 pair-cols vs 8192 cols per
i-tile).  Neg class is sign-flipped so both classes are MAX chains.
A-phases are emitted one group ahead of B-phases so the PE FIFO never
head-of-line blocks on a relu.  4 PSUM groups (2 banks each) rotate.

Host: pos-first sort, exact f64 stats, pairing (self-pair for odd
class tails, fp8-saturated -BIG dummy pad to 4096 pairs, odd columns
peeled into an exact host-side merge), packing, final sqrt/margin/mean
+ exact f64 remainder rows.

Operand layouts (fp8 e4m3):
  e1dr [128, n_it*256]: per i-tile block of 256 cols: [0:128] =
    (-2*e1).T anchor block, [128:256] = ones on partitions 0-3.
  rhsA/rhsB [128, 8192]: per (g, s) chunk of 1024 cols at
    g*2048 + s*1024: [0:512] = pair columns (diff / base side),
    [512:1024] = bias plane (rows 0-3 = 4-term fp8 bias split).
"""

import os
import sys

for _p in ("/opt/trn_rl_repo",):
    if _p not in sys.path:
        sys.path.insert(0, _p)

import numpy as np
import ml_dtypes

EPS = 1e-6
MARGIN = 0.2
B = 8192
D = 128
NCORES = 8
NPAIR = 4096          # pair-columns per core (all cores see all pairs)
GW = 1024             # pair-cols per PSUM group = 2 banks
NG = NPAIR // GW      # 4 groups per i-tile
BIG = 1.0e30

_programs = {}
LAST_RESULTS = None   # BassKernelResults of the most recent run (for profiling)


def _build_program(n_it: int, pairb: int):
    """Bass program for one core.

    n_it: i-tiles (of 128 anchors) per core.
    pairb: pos/neg boundary in pair-column space.
    """
    import concourse.bacc as bacc
    import concourse.tile as tile
    from concourse import mybir

    f32 = mybir.dt.float32
    bf16 = mybir.dt.bfloat16
    fp8 = mybir.dt.float8e4
    AOT = mybir.AluOpType
    AFT = mybir.ActivationFunctionType
    DR = mybir.MatmulPerfMode.DoubleRow

    SH = n_it * 128

    nc = bacc.Bacc(None)
    e1dr = nc.declare_dram_parameter("e1dr", [D, 2 * SH], fp8, isOutput=False)
    rhsAe = nc.declare_dram_parameter("rhsAe", [D, NPAIR], fp8, isOutput=False)
    rhsBe = nc.declare_dram_parameter("rhsBe", [D, NPAIR], fp8, isOutput=False)
    rhsAb = nc.declare_dram_parameter("rhsAb", [4, NPAIR], fp8, isOutput=False)
    rhsBb = nc.declare_dram_parameter("rhsBb", [4, NPAIR], fp8, isOutput=False)
    outp = nc.declare_dram_parameter("out", [128, 2 * n_it], f32, isOutput=True)
    NCH = 2 * NG  # 8 chunks of 512 pair-cols

    def group_segs(g):
        """Class segments (lo, hi, is_pos) of group g in pair-col coords."""
        glo, ghi = g * GW, (g + 1) * GW
        segs = []
        if glo < pairb:
            segs.append((glo, min(ghi, pairb), True))
        if ghi > pairb:
            segs.append((max(glo, pairb), ghi, False))
        return segs

    with tile.TileContext(nc) as tc:
        with (
            tc.tile_pool(name="const", bufs=1) as cpool,
            tc.tile_pool(name="ps", bufs=4, space="PSUM") as pspool,
            tc.tile_pool(name="red", bufs=2) as redpool,
        ):
            e1sb = cpool.tile([D, 2 * SH], fp8, tag="e1sb")
            outsb = cpool.tile([128, 2 * n_it], f32, tag="outsb")
            trf = cpool.tile([128, GW], bf16, tag="trf")
            rhsAsb = cpool.tile([D, 2 * NPAIR], fp8, tag="rhsAsb")
            rhsBsb = cpool.tile([D, 2 * NPAIR], fp8, tag="rhsBsb")

            # Zero the bias-plane garbage rows once (DVE is idle early;
            # rows 4-127 of the i=1 planes multiply against zero weights
            # but must not contain fp8 NaN patterns).
            Av = rhsAsb[:].rearrange("p (c w) -> p c w", c=NCH)
            Bv = rhsBsb[:].rearrange("p (c w) -> p c w", c=NCH)
            Aev = rhsAe[:].rearrange("p (c w) -> p c w", c=NCH)
            Bev = rhsBe[:].rearrange("p (c w) -> p c w", c=NCH)
            Abv = rhsAb[:].rearrange("p (c w) -> p c w", c=NCH)
            Bbv = rhsBb[:].rearrange("p (c w) -> p c w", c=NCH)
            nc.vector.memset(Av[:, :, 512:1024], 0.0)
            nc.vector.memset(Bv[:, :, 512:1024], 0.0)

            # DMA: Act queue stays free for relus; emb planes and 4-row
            # bias planes move separately (skips the zero rows).
            nc.sync.dma_start(Av[:, 0:2, 0:512], Aev[:, 0:2, :])
            nc.gpsimd.dma_start(Bv[:, 0:2, 0:512], Bev[:, 0:2, :])
            nc.sync.dma_start(Av[0:4, :, 512:1024], Abv[:])
            nc.gpsimd.dma_start(Bv[0:4, :, 512:1024], Bbv[:])
            nc.sync.dma_start(e1sb[:], e1dr[:])
            nc.sync.dma_start(Av[:, 2:NCH, 0:512], Aev[:, 2:NCH, :])
            nc.gpsimd.dma_start(Bv[:, 2:NCH, 0:512], Bev[:, 2:NCH, :])

            def emit_A(it, g, ps):
                w3 = e1sb[:, it * 256:(it + 1) * 256].rearrange(
                    "p (i m) -> p i m", i=2
                )
                for s in range(2):
                    c0 = g * 2048 + s * 1024
                    nc.tensor.matmul(
                        ps[:, s * 512:(s + 1) * 512],
                        w3,
                        rhsAsb[:, c0:c0 + 1024].rearrange(
                            "p (i n) -> p i n", i=2
                        ),
                        start=True,
                        stop=True,
                        perf_mode=DR,
                    )
                # relu in place (PSUM -> PSUM, has_written untouched)
                nc.scalar.activation(ps[:], ps[:], AFT.Relu)

            def emit_B(it, g, ps, chain, chain_used):
                w3 = e1sb[:, it * 256:(it + 1) * 256].rearrange(
                    "p (i m) -> p i m", i=2
                )
                for s in range(2):
                    c0 = g * 2048 + s * 1024
                    nc.tensor.matmul(
                        ps[:, s * 512:(s + 1) * 512],
                        w3,
                        rhsBsb[:, c0:c0 + 1024].rearrange(
                            "p (i n) -> p i n", i=2
                        ),
                        start=False,
                        stop=True,
                        perf_mode=DR,
                        skip_group_check=True,
                    )
                # drain: chained max-accum per class segment
                for lo, hi, is_pos in group_segs(g):
                    ll, lh = lo - g * GW, hi - g * GW
                    ci = 0 if is_pos else 1
                    nc.vector.tensor_scalar(
                        out=trf[:, ll:lh],
                        in0=ps[:, ll:lh],
                        scalar1=(chain[:, ci:ci + 1]
                                 if chain_used[is_pos] else -BIG),
                        scalar2=None,
                        op0=AOT.max,
                        op1=AOT.max,
                        accum_out=chain[:, ci:ci + 1],
                    )
                    chain_used[is_pos] = True
                if g == NG - 1:
                    nc.vector.tensor_copy(outsb[:, 2 * it:2 * it + 2], chain[:])

            # Software pipeline, depth 2: A(gen n+1), A(gen n+2) are
            # emitted BEFORE B(gen n) so the PE FIFO never head-of-line
            # blocks on a relu and the DVE stays saturated.
            chains = {}
            pending = []
            for it in range(n_it):
                chains[it] = (
                    redpool.tile([128, 2], f32, tag="chain", name=f"chain_{it}"),
                    {True: False, False: False},
                )
                for g in range(NG):
                    ps = pspool.tile([128, GW], f32, tag="ps", name=f"ps_{it}_{g}")
                    emit_A(it, g, ps)
                    pending.append((it, g, ps))
                    if len(pending) > 2:
                        pit, pg, pps = pending.pop(0)
                        emit_B(pit, pg, pps, *chains[pit])
            for pit, pg, pps in pending:
                emit_B(pit, pg, pps, *chains[pit])
            nc.sync.dma_start(outp[:], outsb[:])
    nc.compile()
    return nc


def _fp8_split4(x):
    """4-term fp8 e4m3 split of x (f64): returns [4, n] planes whose sum
    approximates x to ~1e-3 absolute (saturates at +-448*4)."""
    terms = []
    rem = x.astype(np.float64).copy()
    for _ in range(4):
        t = rem.astype(np.float32).astype(ml_dtypes.float8_e4m3)
        terms.append(t)
        rem = rem - t.astype(np.float64)
    return np.stack(terms)


def _host_prep(emb1, emb2, target):
    """Sort columns pos-first, build pairs, pack device operands."""
    tpos = target == 1
    k = int(tpos.sum())
    perm = np.concatenate([np.nonzero(tpos)[0], np.nonzero(~tpos)[0]])
    e2s = emb2[perm].astype(np.float64)          # [B, D] sorted pos-first
    b = (e2s * e2s).sum(1) - (2.0 * EPS) * e2s.sum(1)

    nneg = B - k
    peel_pos = k % 2
    peel_neg = nneg % 2
    k2, n2 = k - peel_pos, nneg - peel_neg
    peeled = []
    if peel_pos:
        peeled.append((e2s[k - 1], b[k - 1], True))
    if peel_neg:
        peeled.append((e2s[B - 1], b[B - 1], False))

    npairs_pos = k2 // 2
    npairs_neg = n2 // 2
    ndum = NPAIR - npairs_pos - npairs_neg
    assert ndum >= 0

    dA = np.zeros((NPAIR, D))                    # rhsA pair columns (diff)
    dB = np.zeros((NPAIR, D))                    # rhsB pair columns (base)
    bA = np.zeros(NPAIR)
    bB = np.full(NPAIR, -4.0 * 448.0)            # dummy: 4x fp8-saturated
    # pos: A = d_u - d_v ; B = d_v
    u = e2s[0:k2:2]
    v = e2s[1:k2:2]
    dA[:npairs_pos] = u - v
    dB[:npairs_pos] = v
    bA[:npairs_pos] = b[0:k2:2] - b[1:k2:2]
    bB[:npairs_pos] = b[1:k2:2]
    # neg (sign-flipped): A = d_v - d_u ; B = -d_v
    nu = e2s[k:k + n2:2]
    nv = e2s[k + 1:k + n2:2]
    sl = slice(npairs_pos, npairs_pos + npairs_neg)
    dA[sl] = nv - nu
    dB[sl] = -nv
    bA[sl] = b[k + 1:k + n2:2] - b[k:k + n2:2]
    bB[sl] = -b[k + 1:k + n2:2]

    e1p = emb1[tpos]                             # [k, D] pos anchors
    e1d = e1p.astype(np.float64)
    a = (e1d * e1d).sum(1) + (2.0 * EPS) * e1d.sum(1) + D * EPS * EPS

    n_it = min(k // 1024, 8)
    ndev = n_it * 1024
    SH = n_it * 128

    # e1dr: per-core built later (anchor blocks per core); build full here
    e1m2t = (-2.0 * e1p[:ndev]).T.astype(np.float32)   # [D, ndev]

    def pack_rhs(cols, bias):
        colsT = np.ascontiguousarray(cols.T).astype(np.float32).astype(
            ml_dtypes.float8_e4m3)                  # [D, NPAIR]
        bias4 = np.ascontiguousarray(_fp8_split4(bias))  # [4, NPAIR]
        return colsT, bias4

    rhsAe8, rhsAb8 = pack_rhs(dA, bA)
    rhsBe8, rhsBb8 = pack_rhs(dB, bB)
    pairb = npairs_pos
    return (k, n_it, a, e1p, pairb, e1m2t,
            rhsAe8, rhsAb8, rhsBe8, rhsBb8, peeled)


def _pack_e1dr(e1m2t_core, n_it):
    """[D, SH] f32 -> [D, 2*SH] fp8 per-i-tile blocks [emb | ones-rows]."""
    SH = n_it * 128
    out = np.zeros((D, 2 * SH), dtype=ml_dtypes.float8_e4m3)
    for it in range(n_it):
        blk = e1m2t_core[:, it * 128:(it + 1) * 128]
        out[:, it * 256:it * 256 + 128] = blk.astype(ml_dtypes.float8_e4m3)
        out[0:4, it * 256 + 128:it * 256 + 256] = 1.0
    return out


def _host_remainder(e1rem, emb2, target):
    """Exact f64 pos_max/neg_min contribution of the remainder anchors."""
    e1d = e1rem.astype(np.float64)
    e2d = emb2.astype(np.float64)
    sq = (
        (e1d * e1d).sum(1)[:, None]
        + (e2d * e2d).sum(1)[None, :]
        - 2.0 * (e1d @ e2d.T)
        + 2.0 * EPS * (e1d.sum(1)[:, None] - e2d.sum(1)[None, :])
        + D * EPS * EPS
    )
    dist = np.sqrt(np.clip(sq, 0.0, None))
    pos = target == 1
    pos_max = np.where(pos[None, :], dist, -np.inf).max(1)
    neg_min = np.where(~pos[None, :], dist, np.inf).min(1)
    return np.clip(pos_max - neg_min + MARGIN, 0.0, None).sum()


def _numpy_fallback(emb1, emb2, target):
    e1 = emb1.astype(np.float64)
    e2 = emb2.astype(np.float64)
    sq = (
        (e1 * e1).sum(1)[:, None]
        + (e2 * e2).sum(1)[None, :]
        - 2.0 * (e1 @ e2.T)
        + 2.0 * EPS * (e1.sum(1)[:, None] - e2.sum(1)[None, :])
        + D * EPS * EPS
    )
    dist = np.sqrt(np.clip(sq, 0.0, None))
    pos = target == 1
    neg = target == 0
    pos_max = np.where(pos[None, :], dist, -np.inf).max(1)
    neg_min = np.where(neg[None, :], dist, np.inf).min(1)
    per = np.maximum(pos_max - neg_min + MARGIN, 0.0)
    w = pos.astype(np.float64)
    return np.float32((per * w).sum() / w.sum())


def kernel(emb1, emb2, target):
    global LAST_RESULTS
    emb1 = np.asarray(emb1, dtype=np.float32)
    emb2 = np.asarray(emb2, dtype=np.float32)
    target = np.asarray(target)
    assert emb1.shape == (B, D) and emb2.shape == (B, D)

    k = int((target == 1).sum())
    if k < 1024 or k == B:
        return _numpy_fallback(emb1, emb2, target)

    (k, n_it, a, e1p, pairb, e1m2t, rhsAe8, rhsAb8, rhsBe8, rhsBb8,
     peeled) = _host_prep(emb1, emb2, target)
    ndev = n_it * 1024
    SH = n_it * 128

    nc = _programs.get((n_it, pairb))
    if nc is None:
        nc = _build_program(n_it, pairb)
        _programs[(n_it, pairb)] = nc

    from concourse.bass_utils import run_bass_kernel_spmd

    in_maps = [
        {
            "e1dr": _pack_e1dr(e1m2t[:, c * SH:(c + 1) * SH], n_it),
            "rhsAe": rhsAe8,
            "rhsAb": rhsAb8,
            "rhsBe": rhsBe8,
            "rhsBb": rhsBb8,
        }
        for c in range(NCORES)
    ]
    res = run_bass_kernel_spmd(nc, in_maps, core_ids=list(range(NCORES)))
    LAST_RESULTS = res

    Mp = np.concatenate(
        [np.asarray(res.results[c]["out"])[:, 0::2].T.reshape(-1)
         for c in range(NCORES)]
    ).astype(np.float64)
    Mn = np.concatenate(
        [np.asarray(res.results[c]["out"])[:, 1::2].T.reshape(-1)
         for c in range(NCORES)]
    ).astype(np.float64)

    # merge peeled columns exactly (host f64)
    e1d = e1p[:ndev].astype(np.float64)
    for col, bias, is_pos in peeled:
        vj = bias - 2.0 * (e1d @ col)
        if is_pos:
            Mp = np.maximum(Mp, vj)
        else:
            Mn = np.maximum(Mn, -vj)

    adev = a[:ndev]
    pos2 = np.clip(adev + Mp, 0.0, None)
    neg2 = np.clip(adev - Mn, 0.0, None)   # min v = -max(-v)
    per = np.clip(np.sqrt(pos2) - np.sqrt(neg2) + MARGIN, 0.0, None)
    total = per.sum()
    if ndev < k:
        total += _host_remainder(e1p[ndev:], emb2, target)
    return np.float32(total / k)


# revision 12
# speedup vs baseline: 1.4585x; 1.0545x over previous
"""BatchHardTripletLoss kernel for 8 Trainium2 NeuronCores.

Math (matches the jax reference):
  dist2[i,j] = |e1_i|^2 + |e2_j|^2 - 2 e1.e2 + 2*eps*(s1_i - s2_j) + D*eps^2
             = a[i] + v[i,j],   v[i,j] = b[j] - 2<e1_i, e2_j>
  pos_max[i] = sqrt(clip(a[i] + max_{j in pos} v[i,j], 0))
  neg_min[i] = sqrt(clip(a[i] + min_{j in neg} v[i,j], 0))
  loss = mean over POS anchors of relu(pos_max - neg_min + margin)

v3 architecture: PAIRWISE TOURNAMENT + DoubleRow bias folding.
The drain of the [anchors x cands] f32 matrix out of PSUM (DVE/Act at
~1 elem/cyc/partition) is the wall, so candidates are paired on the
host:  max(d_u, d_v) = d_v + relu(d_u - d_v), and d_u - d_v =
(b_u - b_v) - 2<e1, e2_u - e2_v> is ONE matmul column.  Per PSUM
group of 1024 pair-columns:
  phase A: fp8 DoubleRow mains (virtual K=256: 128 embedding dims on
           the i=0 plane, 4-term fp8 bias split on rows 0-3 of the
           i=1 plane against ones in the lhsT) -> diff + bias-diff
  Act:     relu in-place on the PSUM group (only TensorE touches
           has_written, so the accumulate below still works)
  phase B: fp8 DoubleRow mains, start=False -> accumulate base
           d_v (+ its bias) on top of relu(d_u - d_v)
  DVE:     one chained tensor_scalar max-accum per class segment
This halves the reduced stream (4096 pair-cols vs 8192 cols per
i-tile).  Neg class is sign-flipped so both classes are MAX chains.
A-phases are emitted one group ahead of B-phases so the PE FIFO never
head-of-line blocks on a relu.  4 PSUM groups (2 banks each) rotate.

Host: pos-first sort, exact f64 stats, pairing (self-pair for odd
class tails, fp8-saturated -BIG dummy pad to 4096 pairs, odd columns
peeled into an exact host-side merge), packing, final sqrt/margin/mean
+ exact f64 remainder rows.

Operand layouts (fp8 e4m3):
  e1dr [128, n_it*256]: per i-tile block of 256 cols: [0:128] =
    (-2*e1).T anchor block, [128:256] = ones on partitions 0-3.
  rhsA/rhsB [128, 8192]: per (g, s) chunk of 1024 cols at
    g*2048 + s*1024: [0:512] = pair columns (diff / base side),
    [512:1024] = bias plane (rows 0-3 = 4-term fp8 bias split).
"""

import os
import sys

for _p in ("/opt/trn_rl_repo",):
    if _p not in sys.path:
        sys.path.insert(0, _p)

import numpy as np
import ml_dtypes

EPS = 1e-6
MARGIN = 0.2
B = 8192
D = 128
NCORES = 8
NPAIR = 4096          # pair-columns per core (all cores see all pairs)
GW = 1024             # pair-cols per PSUM group = 2 banks
NG = NPAIR // GW      # 4 groups per i-tile
BIG = 1.0e30

_programs = {}
LAST_RESULTS = None   # BassKernelResults of the most recent run (for profiling)


def _build_program(n_it: int, pairb: int):
    """Bass program for one core.

    n_it: i-tiles (of 128 anchors) per core.
    pairb: pos/neg boundary in pair-column space.
    """
    import concourse.bacc as bacc
    import concourse.tile as tile
    from concourse import mybir

    f32 = mybir.dt.float32
    bf16 = mybir.dt.bfloat16
    fp8 = mybir.dt.float8e4
    AOT = mybir.AluOpType
    AFT = mybir.ActivationFunctionType
    DR = mybir.MatmulPerfMode.DoubleRow

    SH = n_it * 128

    nc = bacc.Bacc(None)
    e1dr = nc.declare_dram_parameter("e1dr", [D, 2 * SH], fp8, isOutput=False)
    rhsAe = nc.declare_dram_parameter("rhsAe", [D, NPAIR], fp8, isOutput=False)
    rhsBe = nc.declare_dram_parameter("rhsBe", [D, NPAIR], fp8, isOutput=False)
    rhsAb = nc.declare_dram_parameter("rhsAb", [4, NPAIR], fp8, isOutput=False)
    rhsBb = nc.declare_dram_parameter("rhsBb", [4, NPAIR], fp8, isOutput=False)
    outp = nc.declare_dram_parameter("out", [128, 2 * n_it], f32, isOutput=True)
    NCH = 2 * NG  # 8 chunks of 512 pair-cols

    def group_segs(g):
        """Class segments (lo, hi, is_pos) of group g in pair-col coords."""
        glo, ghi = g * GW, (g + 1) * GW
        segs = []
        if glo < pairb:
            segs.append((glo, min(ghi, pairb), True))
        if ghi > pairb:
            segs.append((max(glo, pairb), ghi, False))
        return segs

    with tile.TileContext(nc) as tc:
        with (
            tc.tile_pool(name="const", bufs=1) as cpool,
            tc.tile_pool(name="ps", bufs=4, space="PSUM") as pspool,
            tc.tile_pool(name="red", bufs=2) as redpool,
        ):
            e1sb = cpool.tile([D, 2 * SH], fp8, tag="e1sb")
            outsb = cpool.tile([128, 2 * n_it], f32, tag="outsb")
            trf = cpool.tile([128, GW], bf16, tag="trf")
            rhsAsb = cpool.tile([D, 2 * NPAIR], fp8, tag="rhsAsb")
            rhsBsb = cpool.tile([D, 2 * NPAIR], fp8, tag="rhsBsb")

            # Zero the bias-plane garbage rows once (rows 4-127 of the
            # i=1 planes multiply against zero weights but must not
            # contain fp8 NaN patterns).  uint32 bitcast -> 4x fewer
            # elements; DVE is idle early.
            Av = rhsAsb[:].rearrange("p (c w) -> p c w", c=NCH)
            Bv = rhsBsb[:].rearrange("p (c w) -> p c w", c=NCH)
            Au = rhsAsb[:].bitcast(mybir.dt.uint32).rearrange(
                "p (c w) -> p c w", c=NCH)
            Bu = rhsBsb[:].bitcast(mybir.dt.uint32).rearrange(
                "p (c w) -> p c w", c=NCH)
            nc.vector.memset(Au[:, :, 128:256], 0)
            nc.vector.memset(Bu[:, :, 128:256], 0)

            Aev = rhsAe[:].rearrange("p (c w) -> p c w", c=NCH)
            Bev = rhsBe[:].rearrange("p (c w) -> p c w", c=NCH)
            Abv = rhsAb[:].rearrange("p (c w) -> p c w", c=NCH)
            Bbv = rhsBb[:].rearrange("p (c w) -> p c w", c=NCH)
            # DMA: emb planes and 4-row bias planes move separately
            # (skips the zero rows); biases on the Act queue (idle early).
            nc.sync.dma_start(e1sb[:], e1dr[:])
            nc.scalar.dma_start(Av[0:4, :, 512:1024], Abv[:])
            nc.sync.dma_start(Av[:, 0:2, 0:512], Aev[:, 0:2, :])
            nc.gpsimd.dma_start(Bv[:, 0:2, 0:512], Bev[:, 0:2, :])
            nc.scalar.dma_start(Bv[0:4, :, 512:1024], Bbv[:])
            nc.sync.dma_start(Av[:, 2:NCH, 0:512], Aev[:, 2:NCH, :])
            nc.gpsimd.dma_start(Bv[:, 2:NCH, 0:512], Bev[:, 2:NCH, :])

            def emit_A(it, g, ps):
                w3 = e1sb[:, it * 256:(it + 1) * 256].rearrange(
                    "p (i m) -> p i m", i=2
                )
                for s in range(2):
                    c0 = g * 2048 + s * 1024
                    nc.tensor.matmul(
                        ps[:, s * 512:(s + 1) * 512],
                        w3,
                        rhsAsb[:, c0:c0 + 1024].rearrange(
                            "p (i n) -> p i n", i=2
                        ),
                        start=True,
                        stop=True,
                        perf_mode=DR,
                    )
                # relu in place (PSUM -> PSUM, has_written untouched)
                nc.scalar.activation(ps[:], ps[:], AFT.Relu)

            def emit_B(it, g, ps, chain, chain_used):
                w3 = e1sb[:, it * 256:(it + 1) * 256].rearrange(
                    "p (i m) -> p i m", i=2
                )
                for s in range(2):
                    c0 = g * 2048 + s * 1024
                    nc.tensor.matmul(
                        ps[:, s * 512:(s + 1) * 512],
                        w3,
                        rhsBsb[:, c0:c0 + 1024].rearrange(
                            "p (i n) -> p i n", i=2
                        ),
                        start=False,
                        stop=True,
                        perf_mode=DR,
                        skip_group_check=True,
                    )
                # drain: chained max-accum per class segment
                for lo, hi, is_pos in group_segs(g):
                    ll, lh = lo - g * GW, hi - g * GW
                    ci = 0 if is_pos else 1
                    nc.vector.tensor_scalar(
                        out=trf[:, ll:lh],
                        in0=ps[:, ll:lh],
                        scalar1=(chain[:, ci:ci + 1]
                                 if chain_used[is_pos] else -BIG),
                        scalar2=None,
                        op0=AOT.max,
                        op1=AOT.max,
                        accum_out=chain[:, ci:ci + 1],
                    )
                    chain_used[is_pos] = True
                if g == NG - 1:
                    nc.vector.tensor_copy(outsb[:, 2 * it:2 * it + 2], chain[:])

            # Software pipeline, depth 2: A(gen n+1), A(gen n+2) are
            # emitted BEFORE B(gen n) so the PE FIFO never head-of-line
            # blocks on a relu and the DVE stays saturated.
            chains = {}
            pending = []
            for it in range(n_it):
                chains[it] = (
                    redpool.tile([128, 2], f32, tag="chain", name=f"chain_{it}"),
                    {True: False, False: False},
                )
                for g in range(NG):
                    ps = pspool.tile([128, GW], f32, tag="ps", name=f"ps_{it}_{g}")
                    emit_A(it, g, ps)
                    pending.append((it, g, ps))
                    if len(pending) > 2:
                        pit, pg, pps = pending.pop(0)
                        emit_B(pit, pg, pps, *chains[pit])
            for pit, pg, pps in pending:
                emit_B(pit, pg, pps, *chains[pit])
            nc.sync.dma_start(outp[:], outsb[:])
    nc.compile()
    return nc


def _fp8_split4(x):
    """4-term fp8 e4m3 split of x (f64): returns [4, n] planes whose sum
    approximates x to ~1e-3 absolute (saturates at +-448*4)."""
    terms = []
    rem = x.astype(np.float64).copy()
    for _ in range(4):
        t = rem.astype(np.float32).astype(ml_dtypes.float8_e4m3)
        terms.append(t)
        rem = rem - t.astype(np.float64)
    return np.stack(terms)


def _host_prep(emb1, emb2, target):
    """Sort columns pos-first, build pairs, pack device operands."""
    tpos = target == 1
    k = int(tpos.sum())
    perm = np.concatenate([np.nonzero(tpos)[0], np.nonzero(~tpos)[0]])
    e2s = emb2[perm].astype(np.float64)          # [B, D] sorted pos-first
    b = (e2s * e2s).sum(1) - (2.0 * EPS) * e2s.sum(1)

    nneg = B - k
    peel_pos = k % 2
    peel_neg = nneg % 2
    k2, n2 = k - peel_pos, nneg - peel_neg
    peeled = []
    if peel_pos:
        peeled.append((e2s[k - 1], b[k - 1], True))
    if peel_neg:
        peeled.append((e2s[B - 1], b[B - 1], False))

    npairs_pos = k2 // 2
    npairs_neg = n2 // 2
    ndum = NPAIR - npairs_pos - npairs_neg
    assert ndum >= 0

    dA = np.zeros((NPAIR, D))                    # rhsA pair columns (diff)
    dB = np.zeros((NPAIR, D))                    # rhsB pair columns (base)
    bA = np.zeros(NPAIR)
    bB = np.full(NPAIR, -4.0 * 448.0)            # dummy: 4x fp8-saturated
    # pos: A = d_u - d_v ; B = d_v
    u = e2s[0:k2:2]
    v = e2s[1:k2:2]
    dA[:npairs_pos] = u - v
    dB[:npairs_pos] = v
    bA[:npairs_pos] = b[0:k2:2] - b[1:k2:2]
    bB[:npairs_pos] = b[1:k2:2]
    # neg (sign-flipped): A = d_v - d_u ; B = -d_v
    nu = e2s[k:k + n2:2]
    nv = e2s[k + 1:k + n2:2]
    sl = slice(npairs_pos, npairs_pos + npairs_neg)
    dA[sl] = nv - nu
    dB[sl] = -nv
    bA[sl] = b[k + 1:k + n2:2] - b[k:k + n2:2]
    bB[sl] = -b[k + 1:k + n2:2]

    e1p = emb1[tpos]                             # [k, D] pos anchors
    e1d = e1p.astype(np.float64)
    a = (e1d * e1d).sum(1) + (2.0 * EPS) * e1d.sum(1) + D * EPS * EPS

    n_it = min(k // 1024, 8)
    ndev = n_it * 1024
    SH = n_it * 128

    # e1dr: per-core built later (anchor blocks per core); build full here
    e1m2t = (-2.0 * e1p[:ndev]).T.astype(np.float32)   # [D, ndev]

    def pack_rhs(cols, bias):
        colsT = np.ascontiguousarray(cols.T).astype(np.float32).astype(
            ml_dtypes.float8_e4m3)                  # [D, NPAIR]
        bias4 = np.ascontiguousarray(_fp8_split4(bias))  # [4, NPAIR]
        return colsT, bias4

    rhsAe8, rhsAb8 = pack_rhs(dA, bA)
    rhsBe8, rhsBb8 = pack_rhs(dB, bB)
    pairb = npairs_pos
    return (k, n_it, a, e1p, pairb, e1m2t,
            rhsAe8, rhsAb8, rhsBe8, rhsBb8, peeled)


def _pack_e1dr(e1m2t_core, n_it):
    """[D, SH] f32 -> [D, 2*SH] fp8 per-i-tile blocks [emb | ones-rows]."""
    SH = n_it * 128
    out = np.zeros((D, 2 * SH), dtype=ml_dtypes.float8_e4m3)
    for it in range(n_it):
        blk = e1m2t_core[:, it * 128:(it + 1) * 128]
        out[:, it * 256:it * 256 + 128] = blk.astype(ml_dtypes.float8_e4m3)
        out[0:4, it * 256 + 128:it * 256 + 256] = 1.0
    return out


def _host_remainder(e1rem, emb2, target):
    """Exact f64 pos_max/neg_min contribution of the remainder anchors."""
    e1d = e1rem.astype(np.float64)
    e2d = emb2.astype(np.float64)
    sq = (
        (e1d * e1d).sum(1)[:, None]
        + (e2d * e2d).sum(1)[None, :]
        - 2.0 * (e1d @ e2d.T)
        + 2.0 * EPS * (e1d.sum(1)[:, None] - e2d.sum(1)[None, :])
        + D * EPS * EPS
    )
    dist = np.sqrt(np.clip(sq, 0.0, None))
    pos = target == 1
    pos_max = np.where(pos[None, :], dist, -np.inf).max(1)
    neg_min = np.where(~pos[None, :], dist, np.inf).min(1)
    return np.clip(pos_max - neg_min + MARGIN, 0.0, None).sum()


def _numpy_fallback(emb1, emb2, target):
    e1 = emb1.astype(np.float64)
    e2 = emb2.astype(np.float64)
    sq = (
        (e1 * e1).sum(1)[:, None]
        + (e2 * e2).sum(1)[None, :]
        - 2.0 * (e1 @ e2.T)
        + 2.0 * EPS * (e1.sum(1)[:, None] - e2.sum(1)[None, :])
        + D * EPS * EPS
    )
    dist = np.sqrt(np.clip(sq, 0.0, None))
    pos = target == 1
    neg = target == 0
    pos_max = np.where(pos[None, :], dist, -np.inf).max(1)
    neg_min = np.where(neg[None, :], dist, np.inf).min(1)
    per = np.maximum(pos_max - neg_min + MARGIN, 0.0)
    w = pos.astype(np.float64)
    return np.float32((per * w).sum() / w.sum())


def kernel(emb1, emb2, target):
    global LAST_RESULTS
    emb1 = np.asarray(emb1, dtype=np.float32)
    emb2 = np.asarray(emb2, dtype=np.float32)
    target = np.asarray(target)
    assert emb1.shape == (B, D) and emb2.shape == (B, D)

    k = int((target == 1).sum())
    if k < 1024 or k == B:
        return _numpy_fallback(emb1, emb2, target)

    (k, n_it, a, e1p, pairb, e1m2t, rhsAe8, rhsAb8, rhsBe8, rhsBb8,
     peeled) = _host_prep(emb1, emb2, target)
    ndev = n_it * 1024
    SH = n_it * 128

    nc = _programs.get((n_it, pairb))
    if nc is None:
        nc = _build_program(n_it, pairb)
        _programs[(n_it, pairb)] = nc

    from concourse.bass_utils import run_bass_kernel_spmd

    in_maps = [
        {
            "e1dr": _pack_e1dr(e1m2t[:, c * SH:(c + 1) * SH], n_it),
            "rhsAe": rhsAe8,
            "rhsAb": rhsAb8,
            "rhsBe": rhsBe8,
            "rhsBb": rhsBb8,
        }
        for c in range(NCORES)
    ]
    res = run_bass_kernel_spmd(nc, in_maps, core_ids=list(range(NCORES)))
    LAST_RESULTS = res

    Mp = np.concatenate(
        [np.asarray(res.results[c]["out"])[:, 0::2].T.reshape(-1)
         for c in range(NCORES)]
    ).astype(np.float64)
    Mn = np.concatenate(
        [np.asarray(res.results[c]["out"])[:, 1::2].T.reshape(-1)
         for c in range(NCORES)]
    ).astype(np.float64)

    # merge peeled columns exactly (host f64)
    e1d = e1p[:ndev].astype(np.float64)
    for col, bias, is_pos in peeled:
        vj = bias - 2.0 * (e1d @ col)
        if is_pos:
            Mp = np.maximum(Mp, vj)
        else:
            Mn = np.maximum(Mn, -vj)

    adev = a[:ndev]
    pos2 = np.clip(adev + Mp, 0.0, None)
    neg2 = np.clip(adev - Mn, 0.0, None)   # min v = -max(-v)
    per = np.clip(np.sqrt(pos2) - np.sqrt(neg2) + MARGIN, 0.0, None)
    total = per.sum()
    if ndev < k:
        total += _host_remainder(e1p[ndev:], emb2, target)
    return np.float32(total / k)
